# revision 7
# baseline (speedup 1.0000x reference)
"""Trainium2 Bass kernel for nn_B_NNs_34789235097695.

Problem: per batch element b (B=262144):
    y   = MLP(s_Ddot[b])  (3 -> 128 -> 128 -> 128 -> 3, tanh, fp32)
    K   = diag geometry from (q[b], s[b])
    A   = 3x3 geometry matrix from (q[b], s[b])
    out = Kdiag * solve(A, y + b3)        -> [B, 3, 1]

Strategy (8 cores, pure data parallel, 32768 batch rows per core):
  - MLP on PE in "hidden-on-partitions" layout: psum = W^T @ xT, chunks of
    1536 batch columns (3 matmuls of N=512 per layer per chunk), fp32r
    (full-rate fp32 mode) for layers 0-2, plain fp32 for the tiny layer 3.
  - tanh on ScalarE reading PSUM directly with fused per-partition bias.
    (ACT is the bottleneck engine: 3*128*32768 = 12.6M tanh/core.)
  - Layer 3 uses h3 slices as the *stationary* operand so the MLP output
    lands batch-on-partitions ([128, 3] per 128-batch slice) — the layout
    the elementwise 3x3 Cramer solve on VectorE wants.
  - Geometry (sin/cos polynomials — q in [0,1) — Kdiag, A, cofactors, det,
    reciprocal) entirely on VectorE in batch-on-partition "order B" layout,
    overlapped under the ACT tanh stream.
  - A small PE-transpose pass converts the MLP output from order A
    (b = f*128 + p) to order B (b = p*256 + f) to meet the geometry layout.
  - q/s/out move as 3KB-contiguous runs; s_Ddot is transposed host-side
    during sharding so layer-0 rhs loads are 3 big runs per chunk.

Self-contained: hardcodes all shapes; needs only /opt/trn_rl_repo (the
container's Bass runtime) and the axon-tunneled NeuronCores.
"""

import sys

for _p in ("/opt/trn_rl_repo", "/root/.axon_site/_ro/trn_rl_repo"):
    if _p not in sys.path:
        sys.path.append(_p)

import numpy as np

B_FULL = 262144
N_CORES = 8
BC = B_FULL // N_CORES          # 32768 batch rows per core
F = BC // 128                   # 256 free columns in geometry layout
H = 128

RB = 0.06                       # BASE_RADIUS
RE = 0.045                      # END_EFFECTOR_RADIUS
LA = 0.176                      # LOWER_ARM_LENGTH

MM_DTYPE = "f32r"               # "f32r" (full-rate) or "f32" (4 cyc/row)

_alpha = np.deg2rad(np.array([-30.0, 90.0, 210.0], np.float32))
CA = [float(v) for v in np.cos(_alpha)]
SA = [float(v) for v in np.sin(_alpha)]

# sin (odd, t=x^2): c1..c9 ; cos (even): d0..d5   -- for q in [0, 1)
_SC = [1.0, -1.0 / 6, 1.0 / 120, -1.0 / 5040, 1.0 / 362880]
_CC = [1.0, -0.5, 1.0 / 24, -1.0 / 720, 1.0 / 40320, -1.0 / 3628800]


def _chunks():
    out = []
    off = 0
    while off < BC:
        size = 1536 if BC - off >= 1536 else BC - off
        out.append((off, size))
        off += size
    return out


def _emit(nc, tc, ctx):
    import concourse.bass as bass
    from concourse import mybir
    from concourse.masks import make_identity

    f32 = mybir.dt.float32
    f32r = mybir.dt.float32r
    ALU = mybir.AluOpType
    ACTF = mybir.ActivationFunctionType

    # dtype used along the fp32r-matmul operand chain (layers 0-2)
    fmm = f32r if MM_DTYPE == "f32r" else f32

    # ---------------- DRAM tensors (per-core shapes) ----------------
    q_d = nc.dram_tensor("q", [BC, 3], f32, kind="ExternalInput").ap()
    s_d = nc.dram_tensor("s", [BC, 3], f32, kind="ExternalInput").ap()
    sddT_d = nc.dram_tensor("sddT", [3, BC], fmm, kind="ExternalInput").ap()
    W_d = [
        nc.dram_tensor("W0", [3, H], fmm, kind="ExternalInput").ap(),
        nc.dram_tensor("W1", [H, H], fmm, kind="ExternalInput").ap(),
        nc.dram_tensor("W2", [H, H], fmm, kind="ExternalInput").ap(),
        nc.dram_tensor("W3", [H, 3], f32, kind="ExternalInput").ap(),
    ]
    b_d = [
        nc.dram_tensor("b0", [H], f32, kind="ExternalInput").ap(),
        nc.dram_tensor("b1", [H], f32, kind="ExternalInput").ap(),
        nc.dram_tensor("b2", [H], f32, kind="ExternalInput").ap(),
        nc.dram_tensor("b3", [3], f32, kind="ExternalInput").ap(),
    ]
    out_d = nc.dram_tensor("out", [BC, 3], f32, kind="ExternalOutput").ap()

    # ---------------- pools ----------------
    singles = ctx.enter_context(tc.tile_pool(name="singles", bufs=1))
    geo = ctx.enter_context(tc.tile_pool(name="geo", bufs=1))
    pool_in = ctx.enter_context(tc.tile_pool(name="pool_in", bufs=3))
    pool_h = ctx.enter_context(tc.tile_pool(name="pool_h", bufs=4))
    psum_mm = ctx.enter_context(tc.tile_pool(name="psum_mm", bufs=2, space="PSUM"))
    psum_sm = ctx.enter_context(tc.tile_pool(name="psum_sm", bufs=2, space="PSUM"))

    # ---------------- constants / weights in SBUF ----------------
    w_sb = []
    for i, wd in enumerate(W_d):
        w = singles.tile(list(wd.shape), wd.dtype, name=f"w{i}sb", tag=f"w{i}sb")
        nc.sync.dma_start(out=w, in_=wd)
        w_sb.append(w)
    b_sb = []
    for i in range(3):
        b = singles.tile([H, 1], f32, name=f"b{i}sb", tag=f"b{i}sb")
        nc.sync.dma_start(out=b, in_=b_d[i].rearrange("(p one) -> p one", one=1))
        b_sb.append(b)
    # b3 broadcast to all partitions: [128, 3]
    b3bc = singles.tile([128, 3], f32, name="b3bc", tag="b3bc")
    nc.gpsimd.dma_start(
        out=b3bc,
        in_=bass.AP(tensor=b_d[3].tensor, offset=0, ap=[[0, 128], [1, 3]]),
    )
    ident = singles.tile([128, 128], f32, name="ident", tag="ident")
    make_identity(nc, ident)

    # interleaved q/s in order B: partition p holds rows [p*F, (p+1)*F)
    iq = singles.tile([128, F, 3], f32, name="iq", tag="iq")
    nc.sync.dma_start(out=iq, in_=q_d.rearrange("(p f) c -> p f c", p=128))
    is_ = singles.tile([128, F, 3], f32, name="is_", tag="is_")
    nc.sync.dma_start(out=is_, in_=s_d.rearrange("(p f) c -> p f c", p=128))

    # MLP output, order A: yintA[p, 3*f + c] = y[f*128 + p, c]
    yintA = singles.tile([128, 3 * F], f32, name="yintA", tag="yintA")

    # ---------------- geometry op list (drained between chunks) ----------
    G = {}  # name -> AP

    def gt(name):
        t = geo.tile([128, F], f32, name=name, tag=name)
        G[name] = t
        return t

    geo_ops = []

    def deferred(fn):
        geo_ops.append(fn)

    vec = nc.vector

    def emit_trig(c):
        x = iq[:, :, c]

        def op_t():
            t = gt(f"t{c}")
            vec.tensor_mul(t, x, x)
        deferred(op_t)

        def op_sin():
            t = G[f"t{c}"]
            c1, c3, c5, c7, c9 = _SC
            w = gt(f"sw{c}")
            vec.scalar_tensor_tensor(w, t, c7 / c9, t, op0=ALU.add, op1=ALU.mult)
            vec.scalar_tensor_tensor(w, w, c5 / c9, t, op0=ALU.add, op1=ALU.mult)
            vec.scalar_tensor_tensor(w, w, c3 / c9, t, op0=ALU.add, op1=ALU.mult)
            vec.tensor_scalar(w, w, c9, 1.0, op0=ALU.mult, op1=ALU.add)
            sq = gt(f"sq{c}")
            vec.tensor_mul(sq, w, x)
        deferred(op_sin)

        def op_cos():
            t = G[f"t{c}"]
            d0, d1, d2, d3, d4, d5 = _CC
            w = gt(f"cw{c}")
            vec.scalar_tensor_tensor(w, t, d4 / d5, t, op0=ALU.add, op1=ALU.mult)
            vec.scalar_tensor_tensor(w, w, d3 / d5, t, op0=ALU.add, op1=ALU.mult)
            vec.scalar_tensor_tensor(w, w, d2 / d5, t, op0=ALU.add, op1=ALU.mult)
            vec.scalar_tensor_tensor(w, w, d1 / d5, t, op0=ALU.add, op1=ALU.mult)
            cq = gt(f"cq{c}")
            vec.tensor_scalar(cq, w, d5, 1.0, op0=ALU.mult, op1=ALU.add)
        deferred(op_cos)

    def emit_kdiag_a(c):
        s0, s1, s2 = is_[:, :, 0], is_[:, :, 1], is_[:, :, 2]

        def op_k():
            sq, cq = G[f"sq{c}"], G[f"cq{c}"]
            u = gt(f"ku{c}")
            vec.tensor_scalar(u, s0, CA[c], RB - RE, op0=ALU.mult, op1=ALU.add)
            vec.scalar_tensor_tensor(u, s1, SA[c], u, op0=ALU.mult, op1=ALU.add)
            vec.tensor_mul(u, u, sq)
            w = gt(f"kw{c}")
            vec.tensor_mul(w, s2, cq)
            k = gt(f"K{c}")
            vec.tensor_sub(k, u, w)
        deferred(op_k)

        def op_a():
            cq = G[f"cq{c}"]
            dR = RE - RB
            a0 = gt(f"a0{c}")
            vec.tensor_scalar(a0, cq, -LA * CA[c], dR * CA[c],
                              op0=ALU.mult, op1=ALU.add)
            vec.tensor_add(a0, a0, s0)
            a1 = gt(f"a1{c}")
            vec.tensor_scalar(a1, cq, -LA * SA[c], dR * SA[c],
                              op0=ALU.mult, op1=ALU.add)
            vec.tensor_add(a1, a1, s1)
            a2 = gt(f"a2{c}")
            vec.scalar_tensor_tensor(a2, cq, -LA, s2, op0=ALU.mult, op1=ALU.add)
        deferred(op_a)

    for c in range(3):
        emit_trig(c)
    for c in range(3):
        emit_kdiag_a(c)

    # cofactors C[i][j] of entry (i,j); adj = C^T ; x_i = sum_j C[j][i]*r_j
    COF = [
        ((0, 0), (1, 1), (2, 2), (1, 2), (2, 1)),
        ((0, 1), (1, 2), (2, 0), (1, 0), (2, 2)),
        ((0, 2), (1, 0), (2, 1), (1, 1), (2, 0)),
        ((1, 0), (0, 2), (2, 1), (0, 1), (2, 2)),
        ((1, 1), (0, 0), (2, 2), (0, 2), (2, 0)),
        ((1, 2), (0, 1), (2, 0), (0, 0), (2, 1)),
        ((2, 0), (0, 1), (1, 2), (0, 2), (1, 1)),
        ((2, 1), (0, 2), (1, 0), (0, 0), (1, 2)),
        ((2, 2), (0, 0), (1, 1), (0, 1), (1, 0)),
    ]

    def emit_cof(spec):
        (ci, cj), (pi, pj), (pk, pl), (ni, nj), (nk, nl) = spec

        def op():
            m1 = gt(f"cm1_{ci}{cj}")
            vec.tensor_mul(m1, G[f"a{pi}{pj}"], G[f"a{pk}{pl}"])
            m2 = gt(f"cm2_{ci}{cj}")
            vec.tensor_mul(m2, G[f"a{ni}{nj}"], G[f"a{nk}{nl}"])
            cc = gt(f"C{ci}{cj}")
            vec.tensor_sub(cc, m1, m2)
        deferred(op)

    for spec in COF:
        emit_cof(spec)

    def op_det():
        m1 = gt("dm1")
        vec.tensor_mul(m1, G["a00"], G["C00"])
        m2 = gt("dm2")
        vec.tensor_mul(m2, G["a01"], G["C01"])
        vec.tensor_add(m1, m1, m2)
        vec.tensor_mul(m2, G["a02"], G["C02"])
        det = gt("det")
        vec.tensor_add(det, m1, m2)
    deferred(op_det)

    def op_rdet():
        rdet = gt("rdet")
        vec.reciprocal(rdet, G["det"])
        for c in range(3):
            krd = gt(f"Krd{c}")
            vec.tensor_mul(krd, G[f"K{c}"], rdet)
    deferred(op_rdet)

    # ---------------- MLP chunks (with geometry drained in between) -----
    chunks = _chunks()
    n_chunks = len(chunks)
    per_gap = (len(geo_ops) + n_chunks - 1) // n_chunks

    sddT_flat = sddT_d  # [3, BC]

    for ci_, (off, S) in enumerate(chunks):
        nS = S // 512
        nJ = S // 128
        fbase = off // 128

        sddc = pool_in.tile([3, S], fmm, name=f"sdd_{ci_}", tag="sdd")
        nc.sync.dma_start(out=sddc, in_=sddT_flat[:, off:off + S])

        ps0 = psum_mm.tile([128, S], f32, name=f"ps0_{ci_}", tag="mm")
        for k in range(nS):
            nc.tensor.matmul(ps0[:, 512 * k:512 * (k + 1)], w_sb[0],
                             sddc[:, 512 * k:512 * (k + 1)],
                             start=True, stop=True)
        h1 = pool_h.tile([128, S], fmm, name=f"h1_{ci_}", tag="h")
        nc.scalar.activation(h1, ps0, ACTF.Tanh, bias=b_sb[0])

        ps1 = psum_mm.tile([128, S], f32, name=f"ps1_{ci_}", tag="mm")
        for k in range(nS):
            nc.tensor.matmul(ps1[:, 512 * k:512 * (k + 1)], w_sb[1],
                             h1[:, 512 * k:512 * (k + 1)],
                             start=True, stop=True)
        h2 = pool_h.tile([128, S], fmm, name=f"h2_{ci_}", tag="h")
        nc.scalar.activation(h2, ps1, ACTF.Tanh, bias=b_sb[1])

        ps2 = psum_mm.tile([128, S], f32, name=f"ps2_{ci_}", tag="mm")
        for k in range(nS):
            nc.tensor.matmul(ps2[:, 512 * k:512 * (k + 1)], w_sb[2],
                             h2[:, 512 * k:512 * (k + 1)],
                             start=True, stop=True)
        h3 = pool_h.tile([128, S], f32, name=f"h3_{ci_}", tag="h")
        nc.scalar.activation(h3, ps2, ACTF.Tanh, bias=b_sb[2])

        # layer 3: h3 slice is the stationary operand; rhs = W3 [128, 3].
        # Output [128, 3] per slice = batch-on-partitions (order A, f=fbase+j).
        l3p = psum_sm.tile([128, 3 * nJ], f32, name=f"l3p_{ci_}", tag="sm")
        for j in range(nJ):
            nc.tensor.matmul(l3p[:, 3 * j:3 * (j + 1)],
                             h3[:, 128 * j:128 * (j + 1)], w_sb[3],
                             start=True, stop=True)
        vec.tensor_copy(yintA[:, 3 * fbase:3 * fbase + 3 * nJ], l3p)

        for _ in range(per_gap):
            if geo_ops:
                geo_ops.pop(0)()

    while geo_ops:
        geo_ops.pop(0)()

    # ---------------- Yint order A -> order B, add b3 -------------------
    # rB_c[p_B, k*128 + p_A] = yA_c[p_A, 2*p_B + k] + b3[c]
    for c in range(3):
        rb = gt(f"r{c}")
        for k in range(2):
            trp = psum_sm.tile([128, 128], f32, name=f"tr{c}{k}", tag="sm")
            nc.tensor.transpose(trp, yintA[:, (c + 3 * k):3 * F:6], ident)
            vec.tensor_scalar(rb[:, 128 * k:128 * (k + 1)], trp,
                              b3bc[:, c:c + 1], None, op0=ALU.add)

    # ---------------- final combine: out = Krd * (C^T r) ----------------
    out_int = singles.tile([128, F, 3], f32, name="out_int", tag="out_int")
    for i in range(3):
        m1 = gt(f"fm1_{i}")
        vec.tensor_mul(m1, G[f"C0{i}"], G["r0"])
        m2 = gt(f"fm2_{i}")
        vec.tensor_mul(m2, G[f"C1{i}"], G["r1"])
        vec.tensor_add(m1, m1, m2)
        vec.tensor_mul(m2, G[f"C2{i}"], G["r2"])
        vec.tensor_add(m1, m1, m2)
        vec.tensor_mul(out_int[:, :, i], m1, G[f"Krd{i}"])

    nc.sync.dma_start(out=out_d.rearrange("(p f) c -> p f c", p=128), in_=out_int)


def build():
    """Build the per-core Bass program (same program for all 8 cores)."""
    from contextlib import ExitStack

    import concourse.bacc as bacc
    import concourse.tile as tile

    nc = bacc.Bacc(trn_type="TRN2", target_bir_lowering=False, debug=False)
    with tile.TileContext(nc) as tc:
        with ExitStack() as ctx:
            _emit(nc, tc, ctx)
    nc.compile()
    return nc


_NC_CACHE = []


def _shard_inputs(inputs):
    f32 = np.float32
    q = np.ascontiguousarray(np.asarray(inputs["q"], dtype=f32))
    s = np.ascontiguousarray(np.asarray(inputs["s"], dtype=f32))
    sdd = np.asarray(inputs["s_Ddot"], dtype=f32)
    weights = {
        k: np.ascontiguousarray(np.asarray(inputs[k], dtype=f32))
        for k in ("W0", "b0", "W1", "b1", "W2", "b2", "W3", "b3")
    }
    in_maps = []
    for c in range(N_CORES):
        sl = slice(c * BC, (c + 1) * BC)
        m = {
            "q": q[sl],
            "s": s[sl],
            "sddT": np.ascontiguousarray(sdd[sl].T),
        }
        m.update(weights)
        in_maps.append(m)
    return in_maps


def kernel(**inputs) -> np.ndarray:
    from concourse import bass_utils

    if not _NC_CACHE:
        _NC_CACHE.append(build())
    nc = _NC_CACHE[0]

    in_maps = _shard_inputs(inputs)
    res = bass_utils.run_bass_kernel_spmd(nc, in_maps, core_ids=list(range(N_CORES)))
    out = np.concatenate([res.results[c]["out"] for c in range(N_CORES)], axis=0)
    return out.reshape(B_FULL, 3, 1).astype(np.float32)


if __name__ == "__main__":
    nc = build()
    print("built OK")


# revision 16
# speedup vs baseline: 1.3163x; 1.3163x over previous
"""Trainium2 Bass kernel for nn_B_NNs_34789235097695.

Problem: per batch element b (B=262144):
    y   = MLP(s_Ddot[b])  (3 -> 128 -> 128 -> 128 -> 3, tanh, fp32)
    K   = diag geometry from (q[b], s[b])
    A   = 3x3 geometry matrix from (q[b], s[b])
    out = Kdiag * solve(A, y + b3)        -> [B, 3, 1]

Strategy (8 cores, pure data parallel, 32768 batch rows per core):
  - MLP on PE in "hidden-on-partitions" layout: psum = W^T @ xT, chunks of
    1536 batch columns (3 matmuls of N=512 per layer per chunk), fp32r
    (full-rate fp32 mode) for layers 0-2, plain fp32 for the tiny layer 3.
  - tanh on ScalarE reading PSUM directly with fused per-partition bias.
    (ACT is the bottleneck engine: 3*128*32768 = 12.6M tanh/core.)
  - Layer 3 uses h3 slices as the *stationary* operand so the MLP output
    lands batch-on-partitions ([128, 3] per 128-batch slice) — the layout
    the elementwise 3x3 Cramer solve on VectorE wants.
  - Geometry (sin/cos polynomials — q in [0,1) — Kdiag, A, cofactors, det,
    reciprocal) entirely on VectorE in batch-on-partition "order B" layout,
    overlapped under the ACT tanh stream.
  - A small PE-transpose pass converts the MLP output from order A
    (b = f*128 + p) to order B (b = p*256 + f) to meet the geometry layout.
  - q/s/out move as 3KB-contiguous runs; s_Ddot is transposed host-side
    during sharding so layer-0 rhs loads are 3 big runs per chunk.

Self-contained: hardcodes all shapes; needs only /opt/trn_rl_repo (the
container's Bass runtime) and the axon-tunneled NeuronCores.
"""

import sys

for _p in ("/opt/trn_rl_repo", "/root/.axon_site/_ro/trn_rl_repo"):
    if _p not in sys.path:
        sys.path.append(_p)

import numpy as np

B_FULL = 262144
N_CORES = 8
BC = B_FULL // N_CORES          # 32768 batch rows per core
F = BC // 128                   # 256 free columns in geometry layout
H = 128

RB = 0.06                       # BASE_RADIUS
RE = 0.045                      # END_EFFECTOR_RADIUS
LA = 0.176                      # LOWER_ARM_LENGTH

MM_DTYPE = "f16"                # "f16" (1 cyc/row) | "f32r" | "f32"

_alpha = np.deg2rad(np.array([-30.0, 90.0, 210.0], np.float32))
CA = [float(v) for v in np.cos(_alpha)]
SA = [float(v) for v in np.sin(_alpha)]

# sin (odd, t=x^2): c1..c9 ; cos (even): d0..d5   -- for q in [0, 1)
_SC = [1.0, -1.0 / 6, 1.0 / 120, -1.0 / 5040, 1.0 / 362880]
_CC = [1.0, -0.5, 1.0 / 24, -1.0 / 720, 1.0 / 40320, -1.0 / 3628800]


def _chunks():
    out = []
    off = 0
    while off < BC:
        size = 1536 if BC - off >= 1536 else BC - off
        out.append((off, size))
        off += size
    return out


def _emit(nc, tc, ctx):
    import concourse.bass as bass
    from concourse import mybir

    f32 = mybir.dt.float32
    ALU = mybir.AluOpType
    ACTF = mybir.ActivationFunctionType

    # dtype used along the matmul operand chain
    fmm = {
        "f16": mybir.dt.float16,
        "f32r": mybir.dt.float32r,
        "f32": f32,
    }[MM_DTYPE]

    # ---------------- DRAM tensors (per-core shapes) ----------------
    q_d = nc.dram_tensor("q", [BC, 3], f32, kind="ExternalInput").ap()
    s_d = nc.dram_tensor("s", [BC, 3], f32, kind="ExternalInput").ap()
    sddT_d = nc.dram_tensor("sddT", [3, BC], fmm, kind="ExternalInput").ap()
    W_d = [
        nc.dram_tensor("W0", [3, H], fmm, kind="ExternalInput").ap(),
        nc.dram_tensor("W1", [H, H], fmm, kind="ExternalInput").ap(),
        nc.dram_tensor("W2", [H, H], fmm, kind="ExternalInput").ap(),
        nc.dram_tensor("W3", [H, 3], fmm, kind="ExternalInput").ap(),
    ]
    b_d = [
        nc.dram_tensor("b0", [H], f32, kind="ExternalInput").ap(),
        nc.dram_tensor("b1", [H], f32, kind="ExternalInput").ap(),
        nc.dram_tensor("b2", [H], f32, kind="ExternalInput").ap(),
        nc.dram_tensor("b3", [3], f32, kind="ExternalInput").ap(),
    ]
    out_d = nc.dram_tensor("out", [BC, 3], f32, kind="ExternalOutput").ap()

    # ---------------- pools ----------------
    singles = ctx.enter_context(tc.tile_pool(name="singles", bufs=1))
    geo = ctx.enter_context(tc.tile_pool(name="geo", bufs=1))
    pool_in = ctx.enter_context(tc.tile_pool(name="pool_in", bufs=3))
    pool_h = ctx.enter_context(tc.tile_pool(name="pool_h", bufs=4))
    pool_stg = ctx.enter_context(tc.tile_pool(name="pool_stg", bufs=3))
    psum_mm = ctx.enter_context(tc.tile_pool(name="psum_mm", bufs=2, space="PSUM"))
    psum_l3 = ctx.enter_context(tc.tile_pool(name="psum_l3", bufs=2, space="PSUM"))

    # ---------------- constants / weights in SBUF ----------------
    w_sb = []
    for i, wd in enumerate(W_d):
        w = singles.tile(list(wd.shape), wd.dtype, name=f"w{i}sb", tag=f"w{i}sb")
        nc.sync.dma_start(out=w, in_=wd)
        w_sb.append(w)
    b_sb = []
    for i in range(3):
        b = singles.tile([H, 1], f32, name=f"b{i}sb", tag=f"b{i}sb")
        nc.sync.dma_start(out=b, in_=b_d[i].rearrange("(p one) -> p one", one=1))
        b_sb.append(b)
    # b3 broadcast to all partitions: [128, 3]
    b3bc = singles.tile([128, 3], f32, name="b3bc", tag="b3bc")
    nc.gpsimd.dma_start(
        out=b3bc,
        in_=bass.AP(tensor=b_d[3].tensor, offset=0, ap=[[0, 128], [1, 3]]),
    )
    # interleaved q/s in order B: partition p holds rows [p*F, (p+1)*F)
    iq = singles.tile([128, F, 3], f32, name="iq", tag="iq")
    nc.sync.dma_start(out=iq, in_=q_d.rearrange("(p f) c -> p f c", p=128))
    is_ = singles.tile([128, F, 3], f32, name="is_", tag="is_")
    nc.sync.dma_start(out=is_, in_=s_d.rearrange("(p f) c -> p f c", p=128))

    # MLP output in order B, comp-major: yB[p, 256*c + f] = y[p*256 + f, c]
    yB = singles.tile([128, 3 * F], f32, name="yB", tag="yB")

    # ---------------- geometry op list (drained between chunks) ----------
    G = {}  # name -> AP

    def gt(name):
        t = geo.tile([128, F], f32, name=name, tag=name)
        G[name] = t
        return t

    geo_ops = []

    def deferred(fn):
        geo_ops.append(fn)

    vec = nc.vector

    def emit_trig(c):
        x = iq[:, :, c]

        def op_t():
            t = gt(f"t{c}")
            vec.tensor_mul(t, x, x)
        deferred(op_t)

        def op_sin():
            t = G[f"t{c}"]
            c1, c3, c5, c7, c9 = _SC
            w = gt(f"sw{c}")
            vec.scalar_tensor_tensor(w, t, c7 / c9, t, op0=ALU.add, op1=ALU.mult)
            vec.scalar_tensor_tensor(w, w, c5 / c9, t, op0=ALU.add, op1=ALU.mult)
            vec.scalar_tensor_tensor(w, w, c3 / c9, t, op0=ALU.add, op1=ALU.mult)
            vec.tensor_scalar(w, w, c9, 1.0, op0=ALU.mult, op1=ALU.add)
            sq = gt(f"sq{c}")
            vec.tensor_mul(sq, w, x)
        deferred(op_sin)

        def op_cos():
            t = G[f"t{c}"]
            d0, d1, d2, d3, d4, d5 = _CC
            w = gt(f"cw{c}")
            vec.scalar_tensor_tensor(w, t, d4 / d5, t, op0=ALU.add, op1=ALU.mult)
            vec.scalar_tensor_tensor(w, w, d3 / d5, t, op0=ALU.add, op1=ALU.mult)
            vec.scalar_tensor_tensor(w, w, d2 / d5, t, op0=ALU.add, op1=ALU.mult)
            vec.scalar_tensor_tensor(w, w, d1 / d5, t, op0=ALU.add, op1=ALU.mult)
            cq = gt(f"cq{c}")
            vec.tensor_scalar(cq, w, d5, 1.0, op0=ALU.mult, op1=ALU.add)
        deferred(op_cos)

    def emit_kdiag_a(c):
        s0, s1, s2 = is_[:, :, 0], is_[:, :, 1], is_[:, :, 2]

        def op_k():
            sq, cq = G[f"sq{c}"], G[f"cq{c}"]
            u = gt(f"ku{c}")
            vec.tensor_scalar(u, s0, CA[c], RB - RE, op0=ALU.mult, op1=ALU.add)
            vec.scalar_tensor_tensor(u, s1, SA[c], u, op0=ALU.mult, op1=ALU.add)
            vec.tensor_mul(u, u, sq)
            w = gt(f"kw{c}")
            vec.tensor_mul(w, s2, cq)
            k = gt(f"K{c}")
            vec.tensor_sub(k, u, w)
        deferred(op_k)

        def op_a():
            cq = G[f"cq{c}"]
            dR = RE - RB
            a0 = gt(f"a0{c}")
            vec.tensor_scalar(a0, cq, -LA * CA[c], dR * CA[c],
                              op0=ALU.mult, op1=ALU.add)
            vec.tensor_add(a0, a0, s0)
            a1 = gt(f"a1{c}")
            vec.tensor_scalar(a1, cq, -LA * SA[c], dR * SA[c],
                              op0=ALU.mult, op1=ALU.add)
            vec.tensor_add(a1, a1, s1)
            a2 = gt(f"a2{c}")
            vec.scalar_tensor_tensor(a2, cq, -LA, s2, op0=ALU.mult, op1=ALU.add)
        deferred(op_a)

    for c in range(3):
        emit_trig(c)
    for c in range(3):
        emit_kdiag_a(c)

    # cofactors C[i][j] of entry (i,j); adj = C^T ; x_i = sum_j C[j][i]*r_j
    COF = [
        ((0, 0), (1, 1), (2, 2), (1, 2), (2, 1)),
        ((0, 1), (1, 2), (2, 0), (1, 0), (2, 2)),
        ((0, 2), (1, 0), (2, 1), (1, 1), (2, 0)),
        ((1, 0), (0, 2), (2, 1), (0, 1), (2, 2)),
        ((1, 1), (0, 0), (2, 2), (0, 2), (2, 0)),
        ((1, 2), (0, 1), (2, 0), (0, 0), (2, 1)),
        ((2, 0), (0, 1), (1, 2), (0, 2), (1, 1)),
        ((2, 1), (0, 2), (1, 0), (0, 0), (1, 2)),
        ((2, 2), (0, 0), (1, 1), (0, 1), (1, 0)),
    ]

    # cofactors on the otherwise-idle GpSimd engine (SBUF-only elementwise)
    def emit_cof(spec):
        (ci, cj), (pi, pj), (pk, pl), (ni, nj), (nk, nl) = spec

        def op():
            gp = nc.gpsimd
            m1 = gt(f"cm1_{ci}{cj}")
            gp.tensor_mul(m1, G[f"a{pi}{pj}"], G[f"a{pk}{pl}"])
            m2 = gt(f"cm2_{ci}{cj}")
            gp.tensor_mul(m2, G[f"a{ni}{nj}"], G[f"a{nk}{nl}"])
            cc = gt(f"C{ci}{cj}")
            gp.tensor_sub(cc, m1, m2)
        deferred(op)

    for spec in COF:
        emit_cof(spec)

    def op_det():
        m1 = gt("dm1")
        vec.tensor_mul(m1, G["a00"], G["C00"])
        m2 = gt("dm2")
        vec.tensor_mul(m2, G["a01"], G["C01"])
        vec.tensor_add(m1, m1, m2)
        vec.tensor_mul(m2, G["a02"], G["C02"])
        det = gt("det")
        vec.tensor_add(det, m1, m2)
    deferred(op_det)

    def op_rdet():
        rdet = gt("rdet")
        vec.reciprocal(rdet, G["det"])
        for c in range(3):
            krd = gt(f"Krd{c}")
            vec.tensor_mul(krd, G[f"K{c}"], rdet)
    deferred(op_rdet)

    # ---------------- MLP chunks (with geometry drained in between) -----
    chunks = _chunks()
    n_chunks = len(chunks)
    per_gap = (len(geo_ops) + n_chunks - 1) // n_chunks

    sddT_flat = sddT_d  # [3, BC]

    for ci_, (off, S) in enumerate(chunks):
        nS = S // 512
        nJ = S // 128

        sddc = pool_in.tile([3, S], fmm, name=f"sdd_{ci_}", tag="sdd")
        nc.sync.dma_start(out=sddc, in_=sddT_flat[:, off:off + S])

        ps0 = psum_mm.tile([128, S], f32, name=f"ps0_{ci_}", tag="mm")
        for k in range(nS):
            nc.tensor.matmul(ps0[:, 512 * k:512 * (k + 1)], w_sb[0],
                             sddc[:, 512 * k:512 * (k + 1)],
                             start=True, stop=True)
        h1 = pool_h.tile([128, S], fmm, name=f"h1_{ci_}", tag="h")
        nc.scalar.activation(h1, ps0, ACTF.Tanh, bias=b_sb[0])

        ps1 = psum_mm.tile([128, S], f32, name=f"ps1_{ci_}", tag="mm")
        for k in range(nS):
            nc.tensor.matmul(ps1[:, 512 * k:512 * (k + 1)], w_sb[1],
                             h1[:, 512 * k:512 * (k + 1)],
                             start=True, stop=True)
        h2 = pool_h.tile([128, S], fmm, name=f"h2_{ci_}", tag="h")
        nc.scalar.activation(h2, ps1, ACTF.Tanh, bias=b_sb[1])

        ps2 = psum_mm.tile([128, S], f32, name=f"ps2_{ci_}", tag="mm")
        for k in range(nS):
            nc.tensor.matmul(ps2[:, 512 * k:512 * (k + 1)], w_sb[2],
                             h2[:, 512 * k:512 * (k + 1)],
                             start=True, stop=True)
        h3 = pool_h.tile([128, S], fmm, name=f"h3_{ci_}", tag="h")
        nc.scalar.activation(h3, ps2, ACTF.Tanh, bias=b_sb[2])

        # layer 3: lhsT = W3 [128, 3] (3-column weight load), rhs = h3.
        # Output [3, 512] per sub-chunk, copied out by DVE then reshaped to
        # order B (batch-on-partitions) with a small SBUF->SBUF DMA.
        stg = pool_stg.tile([3, S], f32, name=f"stg_{ci_}", tag="stg")
        for k in range(nS):
            psl3 = psum_l3.tile([3, 512], f32, name=f"l3_{ci_}_{k}", tag="l3")
            nc.tensor.matmul(psl3, w_sb[3], h3[:, 512 * k:512 * (k + 1)],
                             start=True, stop=True)
            vec.tensor_copy(stg[:, 512 * k:512 * (k + 1)], psl3)
        nP = S // F                       # partition rows covered (6 or 2)
        p0 = off // F
        for c in range(3):
            nc.sync.dma_start(
                out=yB[p0:p0 + nP, F * c:F * (c + 1)],
                in_=stg[c:c + 1, :].rearrange("one (p f) -> one p f", f=F),
            )

        for _ in range(per_gap):
            if geo_ops:
                geo_ops.pop(0)()

    while geo_ops:
        geo_ops.pop(0)()

    # ---------------- r_c = yB_c + b3[c] --------------------------------
    for c in range(3):
        rb = gt(f"r{c}")
        vec.tensor_scalar(rb, yB[:, F * c:F * (c + 1)], b3bc[:, c:c + 1],
                          None, op0=ALU.add)

    # ---------------- final combine: out = Krd * (C^T r) ----------------
    out_int = singles.tile([128, F, 3], f32, name="out_int", tag="out_int")
    for i in range(3):
        m1 = gt(f"fm1_{i}")
        vec.tensor_mul(m1, G[f"C0{i}"], G["r0"])
        m2 = gt(f"fm2_{i}")
        vec.tensor_mul(m2, G[f"C1{i}"], G["r1"])
        vec.tensor_add(m1, m1, m2)
        vec.tensor_mul(m2, G[f"C2{i}"], G["r2"])
        vec.tensor_add(m1, m1, m2)
        vec.tensor_mul(out_int[:, :, i], m1, G[f"Krd{i}"])

    nc.sync.dma_start(out=out_d.rearrange("(p f) c -> p f c", p=128), in_=out_int)


def build():
    """Build the per-core Bass program (same program for all 8 cores)."""
    from contextlib import ExitStack

    import concourse.bacc as bacc
    import concourse.tile as tile

    nc = bacc.Bacc(trn_type="TRN2", target_bir_lowering=False, debug=False)
    with tile.TileContext(nc) as tc:
        with ExitStack() as ctx:
            _emit(nc, tc, ctx)
    nc.compile()
    return nc


_NC_CACHE = []


def _shard_inputs(inputs):
    f32 = np.float32
    fmm = {"f16": np.float16, "f32r": f32, "f32": f32}[MM_DTYPE]
    q = np.ascontiguousarray(np.asarray(inputs["q"], dtype=f32))
    s = np.ascontiguousarray(np.asarray(inputs["s"], dtype=f32))
    sdd = np.asarray(inputs["s_Ddot"], dtype=f32)
    weights = {}
    for k in ("W0", "W1", "W2", "W3"):
        weights[k] = np.ascontiguousarray(np.asarray(inputs[k], dtype=f32).astype(fmm))
    for k in ("b0", "b1", "b2", "b3"):
        weights[k] = np.ascontiguousarray(np.asarray(inputs[k], dtype=f32))
    in_maps = []
    for c in range(N_CORES):
        sl = slice(c * BC, (c + 1) * BC)
        m = {
            "q": q[sl],
            "s": s[sl],
            "sddT": np.ascontiguousarray(sdd[sl].T).astype(fmm),
        }
        m.update(weights)
        in_maps.append(m)
    return in_maps


def kernel(**inputs) -> np.ndarray:
    from concourse import bass_utils

    if not _NC_CACHE:
        _NC_CACHE.append(build())
    nc = _NC_CACHE[0]

    in_maps = _shard_inputs(inputs)
    res = bass_utils.run_bass_kernel_spmd(nc, in_maps, core_ids=list(range(N_CORES)))
    out = np.concatenate([res.results[c]["out"] for c in range(N_CORES)], axis=0)
    return out.reshape(B_FULL, 3, 1).astype(np.float32)


if __name__ == "__main__":
    nc = build()
    print("built OK")


# revision 19
# speedup vs baseline: 1.8544x; 1.4088x over previous
"""Trainium2 Bass kernel for nn_B_NNs_34789235097695.

Problem: per batch element b (B=262144):
    y   = MLP(s_Ddot[b])  (3 -> 128 -> 128 -> 128 -> 3, tanh, fp32)
    K   = diag geometry from (q[b], s[b])
    A   = 3x3 geometry matrix from (q[b], s[b])
    out = Kdiag * solve(A, y + b3)        -> [B, 3, 1]

Strategy (8 cores, pure data parallel, 32768 batch rows per core):
  - MLP on PE in "hidden-on-partitions" layout: psum = W^T @ xT, chunks of
    1536 batch columns (3 matmuls of N=512 per layer per chunk), fp32r
    (full-rate fp32 mode) for layers 0-2, plain fp32 for the tiny layer 3.
  - tanh on ScalarE reading PSUM directly with fused per-partition bias.
    (ACT is the bottleneck engine: 3*128*32768 = 12.6M tanh/core.)
  - Layer 3 uses h3 slices as the *stationary* operand so the MLP output
    lands batch-on-partitions ([128, 3] per 128-batch slice) — the layout
    the elementwise 3x3 Cramer solve on VectorE wants.
  - Geometry (sin/cos polynomials — q in [0,1) — Kdiag, A, cofactors, det,
    reciprocal) entirely on VectorE in batch-on-partition "order B" layout,
    overlapped under the ACT tanh stream.
  - A small PE-transpose pass converts the MLP output from order A
    (b = f*128 + p) to order B (b = p*256 + f) to meet the geometry layout.
  - q/s/out move as 3KB-contiguous runs; s_Ddot is transposed host-side
    during sharding so layer-0 rhs loads are 3 big runs per chunk.

Self-contained: hardcodes all shapes; needs only /opt/trn_rl_repo (the
container's Bass runtime) and the axon-tunneled NeuronCores.
"""

import sys

for _p in ("/opt/trn_rl_repo", "/root/.axon_site/_ro/trn_rl_repo"):
    if _p not in sys.path:
        sys.path.append(_p)

import numpy as np

B_FULL = 262144
N_CORES = 8
BC = B_FULL // N_CORES          # 32768 batch rows per core
F = BC // 128                   # 256 free columns in geometry layout
H = 128

RB = 0.06                       # BASE_RADIUS
RE = 0.045                      # END_EFFECTOR_RADIUS
LA = 0.176                      # LOWER_ARM_LENGTH

MM_DTYPE = "f16"                # "f16" (1 cyc/row) | "f32r" | "f32"

_alpha = np.deg2rad(np.array([-30.0, 90.0, 210.0], np.float32))
CA = [float(v) for v in np.cos(_alpha)]
SA = [float(v) for v in np.sin(_alpha)]

# sin (odd, t=x^2): c1..c9 ; cos (even): d0..d5   -- for q in [0, 1)
_SC = [1.0, -1.0 / 6, 1.0 / 120, -1.0 / 5040, 1.0 / 362880]
_CC = [1.0, -0.5, 1.0 / 24, -1.0 / 720, 1.0 / 40320, -1.0 / 3628800]


CHUNK = 1024                    # 2 PSUM banks per stage tile


def _chunks():
    assert BC % CHUNK == 0
    return [(i * CHUNK, CHUNK) for i in range(BC // CHUNK)]


def _emit(nc, tc, ctx):
    import concourse.bass as bass
    from concourse import mybir

    f32 = mybir.dt.float32
    ALU = mybir.AluOpType
    ACTF = mybir.ActivationFunctionType

    # dtype used along the matmul operand chain
    fmm = {
        "f16": mybir.dt.float16,
        "f32r": mybir.dt.float32r,
        "f32": f32,
    }[MM_DTYPE]

    # ---------------- DRAM tensors (per-core shapes) ----------------
    q_d = nc.dram_tensor("q", [BC, 3], f32, kind="ExternalInput").ap()
    s_d = nc.dram_tensor("s", [BC, 3], f32, kind="ExternalInput").ap()
    sddT_d = nc.dram_tensor("sddT", [3, BC], fmm, kind="ExternalInput").ap()
    W_d = [
        nc.dram_tensor("W0", [3, H], fmm, kind="ExternalInput").ap(),
        nc.dram_tensor("W1", [H, H], fmm, kind="ExternalInput").ap(),
        nc.dram_tensor("W2", [H, H], fmm, kind="ExternalInput").ap(),
        nc.dram_tensor("W3", [H, 3], fmm, kind="ExternalInput").ap(),
    ]
    b_d = [
        nc.dram_tensor("b0", [H], f32, kind="ExternalInput").ap(),
        nc.dram_tensor("b1", [H], f32, kind="ExternalInput").ap(),
        nc.dram_tensor("b2", [H], f32, kind="ExternalInput").ap(),
        nc.dram_tensor("b3", [3], f32, kind="ExternalInput").ap(),
    ]
    out_d = nc.dram_tensor("out", [BC, 3], f32, kind="ExternalOutput").ap()

    # ---------------- pools ----------------
    singles = ctx.enter_context(tc.tile_pool(name="singles", bufs=1))
    geo = ctx.enter_context(tc.tile_pool(name="geo", bufs=1))
    pool_in = ctx.enter_context(tc.tile_pool(name="pool_in", bufs=4))
    pool_h = ctx.enter_context(tc.tile_pool(name="pool_h", bufs=6))
    pool_stg = ctx.enter_context(tc.tile_pool(name="pool_stg", bufs=3))
    # 3 stage tags (mm0/mm1/mm2) x 1 buf x 2 banks + l3 2 x 1 bank = 8 banks
    psum_mm = ctx.enter_context(tc.tile_pool(name="psum_mm", bufs=1, space="PSUM"))
    psum_l3 = ctx.enter_context(tc.tile_pool(name="psum_l3", bufs=2, space="PSUM"))

    # ---------------- constants / weights in SBUF ----------------
    w_sb = []
    for i, wd in enumerate(W_d):
        w = singles.tile(list(wd.shape), wd.dtype, name=f"w{i}sb", tag=f"w{i}sb")
        nc.sync.dma_start(out=w, in_=wd)
        w_sb.append(w)
    b_sb = []
    for i in range(3):
        b = singles.tile([H, 1], f32, name=f"b{i}sb", tag=f"b{i}sb")
        nc.sync.dma_start(out=b, in_=b_d[i].rearrange("(p one) -> p one", one=1))
        b_sb.append(b)
    # b3 broadcast to all partitions: [128, 3]
    b3bc = singles.tile([128, 3], f32, name="b3bc", tag="b3bc")
    nc.gpsimd.dma_start(
        out=b3bc,
        in_=bass.AP(tensor=b_d[3].tensor, offset=0, ap=[[0, 128], [1, 3]]),
    )
    # interleaved q/s in order B: partition p holds rows [p*F, (p+1)*F)
    iq = singles.tile([128, F, 3], f32, name="iq", tag="iq")
    nc.sync.dma_start(out=iq, in_=q_d.rearrange("(p f) c -> p f c", p=128))
    is_ = singles.tile([128, F, 3], f32, name="is_", tag="is_")
    nc.sync.dma_start(out=is_, in_=s_d.rearrange("(p f) c -> p f c", p=128))

    # MLP output in order B, comp-major: yB[p, 256*c + f] = y[p*256 + f, c]
    yB = singles.tile([128, 3 * F], f32, name="yB", tag="yB")

    # ---------------- geometry op list (drained between chunks) ----------
    G = {}  # name -> AP

    def gt(name):
        t = geo.tile([128, F], f32, name=name, tag=name)
        G[name] = t
        return t

    geo_ops = []

    def deferred(fn):
        geo_ops.append(fn)

    vec = nc.vector

    def emit_trig(c):
        x = iq[:, :, c]

        def op_t():
            t = gt(f"t{c}")
            vec.tensor_mul(t, x, x)
        deferred(op_t)

        def op_sin():
            t = G[f"t{c}"]
            c1, c3, c5, c7, c9 = _SC
            w = gt(f"sw{c}")
            vec.scalar_tensor_tensor(w, t, c7 / c9, t, op0=ALU.add, op1=ALU.mult)
            vec.scalar_tensor_tensor(w, w, c5 / c9, t, op0=ALU.add, op1=ALU.mult)
            vec.scalar_tensor_tensor(w, w, c3 / c9, t, op0=ALU.add, op1=ALU.mult)
            vec.tensor_scalar(w, w, c9, 1.0, op0=ALU.mult, op1=ALU.add)
            sq = gt(f"sq{c}")
            vec.tensor_mul(sq, w, x)
        deferred(op_sin)

        def op_cos():
            t = G[f"t{c}"]
            d0, d1, d2, d3, d4, d5 = _CC
            w = gt(f"cw{c}")
            vec.scalar_tensor_tensor(w, t, d4 / d5, t, op0=ALU.add, op1=ALU.mult)
            vec.scalar_tensor_tensor(w, w, d3 / d5, t, op0=ALU.add, op1=ALU.mult)
            vec.scalar_tensor_tensor(w, w, d2 / d5, t, op0=ALU.add, op1=ALU.mult)
            vec.scalar_tensor_tensor(w, w, d1 / d5, t, op0=ALU.add, op1=ALU.mult)
            cq = gt(f"cq{c}")
            vec.tensor_scalar(cq, w, d5, 1.0, op0=ALU.mult, op1=ALU.add)
        deferred(op_cos)

    def emit_kdiag_a(c):
        s0, s1, s2 = is_[:, :, 0], is_[:, :, 1], is_[:, :, 2]

        def op_k():
            sq, cq = G[f"sq{c}"], G[f"cq{c}"]
            u = gt(f"ku{c}")
            vec.tensor_scalar(u, s0, CA[c], RB - RE, op0=ALU.mult, op1=ALU.add)
            vec.scalar_tensor_tensor(u, s1, SA[c], u, op0=ALU.mult, op1=ALU.add)
            vec.tensor_mul(u, u, sq)
            w = gt(f"kw{c}")
            vec.tensor_mul(w, s2, cq)
            k = gt(f"K{c}")
            vec.tensor_sub(k, u, w)
        deferred(op_k)

        def op_a():
            cq = G[f"cq{c}"]
            dR = RE - RB
            a0 = gt(f"a0{c}")
            vec.tensor_scalar(a0, cq, -LA * CA[c], dR * CA[c],
                              op0=ALU.mult, op1=ALU.add)
            vec.tensor_add(a0, a0, s0)
            a1 = gt(f"a1{c}")
            vec.tensor_scalar(a1, cq, -LA * SA[c], dR * SA[c],
                              op0=ALU.mult, op1=ALU.add)
            vec.tensor_add(a1, a1, s1)
            a2 = gt(f"a2{c}")
            vec.scalar_tensor_tensor(a2, cq, -LA, s2, op0=ALU.mult, op1=ALU.add)
        deferred(op_a)

    for c in range(3):
        emit_trig(c)
    for c in range(3):
        emit_kdiag_a(c)

    # cofactors C[i][j] of entry (i,j); adj = C^T ; x_i = sum_j C[j][i]*r_j
    COF = [
        ((0, 0), (1, 1), (2, 2), (1, 2), (2, 1)),
        ((0, 1), (1, 2), (2, 0), (1, 0), (2, 2)),
        ((0, 2), (1, 0), (2, 1), (1, 1), (2, 0)),
        ((1, 0), (0, 2), (2, 1), (0, 1), (2, 2)),
        ((1, 1), (0, 0), (2, 2), (0, 2), (2, 0)),
        ((1, 2), (0, 1), (2, 0), (0, 0), (2, 1)),
        ((2, 0), (0, 1), (1, 2), (0, 2), (1, 1)),
        ((2, 1), (0, 2), (1, 0), (0, 0), (1, 2)),
        ((2, 2), (0, 0), (1, 1), (0, 1), (1, 0)),
    ]

    # cofactors on the otherwise-idle GpSimd engine (SBUF-only elementwise)
    def emit_cof(spec):
        (ci, cj), (pi, pj), (pk, pl), (ni, nj), (nk, nl) = spec

        def op():
            gp = nc.gpsimd
            m1 = gt(f"cm1_{ci}{cj}")
            gp.tensor_mul(m1, G[f"a{pi}{pj}"], G[f"a{pk}{pl}"])
            m2 = gt(f"cm2_{ci}{cj}")
            gp.tensor_mul(m2, G[f"a{ni}{nj}"], G[f"a{nk}{nl}"])
            cc = gt(f"C{ci}{cj}")
            gp.tensor_sub(cc, m1, m2)
        deferred(op)

    for spec in COF:
        emit_cof(spec)

    def op_det():
        m1 = gt("dm1")
        vec.tensor_mul(m1, G["a00"], G["C00"])
        m2 = gt("dm2")
        vec.tensor_mul(m2, G["a01"], G["C01"])
        vec.tensor_add(m1, m1, m2)
        vec.tensor_mul(m2, G["a02"], G["C02"])
        det = gt("det")
        vec.tensor_add(det, m1, m2)
    deferred(op_det)

    def op_rdet():
        rdet = gt("rdet")
        vec.reciprocal(rdet, G["det"])
        for c in range(3):
            krd = gt(f"Krd{c}")
            vec.tensor_mul(krd, G[f"K{c}"], rdet)
    deferred(op_rdet)

    # ---------------- MLP chunks: 3-stage skewed software pipeline -------
    # ACT is the bottleneck engine and its queue is strict-FIFO, so tanh
    # instructions are emitted in the order T0(i), T1(i-1), T2(i-2): the
    # PE work between dependent tanhs of one chunk is hidden under the
    # other chunks' tanhs, keeping ACT (and PE, for HAM warmth) dense.
    chunks = _chunks()
    n_chunks = len(chunks)
    n_iters = n_chunks + 2
    per_gap = (len(geo_ops) + n_iters - 1) // n_iters

    PS = {}   # (stage, chunk) -> psum tile
    HT = {}   # (stage, chunk) -> h tile

    def st_dma(ci):
        off, S = chunks[ci]
        sddc = pool_in.tile([3, S], fmm, name=f"sdd_{ci}", tag="sdd")
        nc.sync.dma_start(out=sddc, in_=sddT_d[:, off:off + S])
        HT[("x", ci)] = sddc

    def st_mm(layer, ci):
        _, S = chunks[ci]
        nS = S // 512
        src = HT[("x", ci)] if layer == 0 else HT[(layer - 1, ci)]
        ps = psum_mm.tile([128, S], f32, name=f"ps{layer}_{ci}",
                          tag=f"mm{layer}")
        for k in range(nS):
            nc.tensor.matmul(ps[:, 512 * k:512 * (k + 1)], w_sb[layer],
                             src[:, 512 * k:512 * (k + 1)],
                             start=True, stop=True)
        PS[(layer, ci)] = ps

    def st_tanh(layer, ci):
        _, S = chunks[ci]
        h = pool_h.tile([128, S], fmm, name=f"h{layer}_{ci}", tag="h")
        nc.scalar.activation(h, PS[(layer, ci)], ACTF.Tanh, bias=b_sb[layer])
        HT[(layer, ci)] = h
        del PS[(layer, ci)]

    def st_l3(ci):
        off, S = chunks[ci]
        nS = S // 512
        h3 = HT[(2, ci)]
        stg = pool_stg.tile([3, S], f32, name=f"stg_{ci}", tag="stg")
        for k in range(nS):
            psl3 = psum_l3.tile([3, 512], f32, name=f"l3_{ci}_{k}", tag="l3")
            nc.tensor.matmul(psl3, w_sb[3], h3[:, 512 * k:512 * (k + 1)],
                             start=True, stop=True)
            vec.tensor_copy(stg[:, 512 * k:512 * (k + 1)], psl3)
        nP = S // F
        p0 = off // F
        for c in range(3):
            nc.sync.dma_start(
                out=yB[p0:p0 + nP, F * c:F * (c + 1)],
                in_=stg[c:c + 1, :].rearrange("one (p f) -> one p f", f=F),
            )

    st_dma(0)
    st_mm(0, 0)
    st_dma(1)
    for i in range(n_iters):
        if i + 2 < n_chunks:
            st_dma(i + 2)
        if i + 1 < n_chunks:
            st_mm(0, i + 1)
        if i < n_chunks:
            st_tanh(0, i)
            st_mm(1, i)
        if 0 <= i - 1 < n_chunks:
            st_tanh(1, i - 1)
            st_mm(2, i - 1)
        if 0 <= i - 2 < n_chunks:
            st_tanh(2, i - 2)
            st_l3(i - 2)
        for _ in range(per_gap):
            if geo_ops:
                geo_ops.pop(0)()

    while geo_ops:
        geo_ops.pop(0)()

    # ---------------- r_c = yB_c + b3[c] --------------------------------
    for c in range(3):
        rb = gt(f"r{c}")
        vec.tensor_scalar(rb, yB[:, F * c:F * (c + 1)], b3bc[:, c:c + 1],
                          None, op0=ALU.add)

    # ---------------- final combine: out = Krd * (C^T r) ----------------
    out_int = singles.tile([128, F, 3], f32, name="out_int", tag="out_int")
    for i in range(3):
        m1 = gt(f"fm1_{i}")
        vec.tensor_mul(m1, G[f"C0{i}"], G["r0"])
        m2 = gt(f"fm2_{i}")
        vec.tensor_mul(m2, G[f"C1{i}"], G["r1"])
        vec.tensor_add(m1, m1, m2)
        vec.tensor_mul(m2, G[f"C2{i}"], G["r2"])
        vec.tensor_add(m1, m1, m2)
        vec.tensor_mul(out_int[:, :, i], m1, G[f"Krd{i}"])

    nc.sync.dma_start(out=out_d.rearrange("(p f) c -> p f c", p=128), in_=out_int)


def build():
    """Build the per-core Bass program (same program for all 8 cores)."""
    from contextlib import ExitStack

    import concourse.bacc as bacc
    import concourse.tile as tile

    nc = bacc.Bacc(trn_type="TRN2", target_bir_lowering=False, debug=False)
    with tile.TileContext(nc) as tc:
        with ExitStack() as ctx:
            _emit(nc, tc, ctx)
    nc.compile()
    return nc


_NC_CACHE = []


def _shard_inputs(inputs):
    f32 = np.float32
    fmm = {"f16": np.float16, "f32r": f32, "f32": f32}[MM_DTYPE]
    q = np.ascontiguousarray(np.asarray(inputs["q"], dtype=f32))
    s = np.ascontiguousarray(np.asarray(inputs["s"], dtype=f32))
    sdd = np.asarray(inputs["s_Ddot"], dtype=f32)
    weights = {}
    for k in ("W0", "W1", "W2", "W3"):
        weights[k] = np.ascontiguousarray(np.asarray(inputs[k], dtype=f32).astype(fmm))
    for k in ("b0", "b1", "b2", "b3"):
        weights[k] = np.ascontiguousarray(np.asarray(inputs[k], dtype=f32))
    in_maps = []
    for c in range(N_CORES):
        sl = slice(c * BC, (c + 1) * BC)
        m = {
            "q": q[sl],
            "s": s[sl],
            "sddT": np.ascontiguousarray(sdd[sl].T).astype(fmm),
        }
        m.update(weights)
        in_maps.append(m)
    return in_maps


def kernel(**inputs) -> np.ndarray:
    from concourse import bass_utils

    if not _NC_CACHE:
        _NC_CACHE.append(build())
    nc = _NC_CACHE[0]

    in_maps = _shard_inputs(inputs)
    res = bass_utils.run_bass_kernel_spmd(nc, in_maps, core_ids=list(range(N_CORES)))
    out = np.concatenate([res.results[c]["out"] for c in range(N_CORES)], axis=0)
    return out.reshape(B_FULL, 3, 1).astype(np.float32)


if __name__ == "__main__":
    nc = build()
    print("built OK")
